# revision 1
# baseline (speedup 1.0000x reference)
"""Trainium2 Bass kernel for DBFLinear:
    y = ((x * s0) @ unpack(bp1).T * s2) @ unpack(bp3).T * s4 + bias

Strategy: data-parallel over batch across 8 cores (weights replicated, no
collectives). Per core: unpack the bit-packed +/-1 weights on device
(DVE bitwise_and + ACT Sign), transpose weight blocks with the DMA xbar,
run both GEMMs weight-stationary (fp16, fp32 PSUM accumulation). scaling0
is folded into the unpacked W1 (+/-s0 is exact in fp16), scaling2 into the
h eviction, scaling4+bias into the y eviction — all per-partition ACT ops.
The device emits y.T per batch shard; the host transposes while unsharding.
"""

import sys

import numpy as np

sys.path.insert(0, "/opt/trn_rl_repo")

import concourse.bass as bass
import concourse.mybir as mybir
import concourse.tile as tile
from concourse.tile import add_dep_helper
from concourse import bacc
from concourse.bass_utils import run_bass_kernel_spmd

N_CORES = 8
B_FULL, IN, MID, OUT = 8192, 4096, 4096, 4096
P = 128
FD = 512  # matmul moving-operand free dim (1 PSUM bank of fp32)
QCH = 1024  # unpack quarter width (weight elements per DVE/ACT op)
N_WARM = 800  # HAM warm-up matmuls


def build_program(b=B_FULL // N_CORES, in_=IN, mid=MID, out=OUT):
    """Build the per-core Bass program. Returns the Bass object."""
    in_k, mid_k, out_k = in_ // P, mid // P, out // P
    nbc = 2  # batch processed as two halves
    fd = b // nbc
    assert fd <= FD, (b, fd)
    uch = min(QCH, in_, mid)

    nc = bacc.Bacc(num_devices=N_CORES)
    x_d = nc.dram_tensor("x", [b, in_], mybir.dt.float16, kind="ExternalInput")
    bp1_d = nc.dram_tensor("bp1", [mid, in_ // 8], mybir.dt.int32, kind="ExternalInput")
    bp3_d = nc.dram_tensor("bp3", [out, mid // 8], mybir.dt.int32, kind="ExternalInput")
    mask_d = nc.dram_tensor("mask", [P, 8], mybir.dt.int32, kind="ExternalInput")
    s0r_d = nc.dram_tensor("s0rep", [P, in_], mybir.dt.float16, kind="ExternalInput")
    s2_d = nc.dram_tensor("s2", [P, mid_k], mybir.dt.float32, kind="ExternalInput")
    s4_d = nc.dram_tensor("s4", [P, out_k], mybir.dt.float32, kind="ExternalInput")
    bias_d = nc.dram_tensor("bias", [P, out_k], mybir.dt.float32, kind="ExternalInput")
    yT_d = nc.dram_tensor("yT", [out, b], mybir.dt.float16, kind="ExternalOutput")

    Act = mybir.ActivationFunctionType

    with tile.TileContext(nc) as tc:
        with (
            tc.tile_pool(name="big", bufs=1) as big,
            tc.tile_pool(name="consts", bufs=1) as consts,
            tc.tile_pool(name="wpipe", bufs=2) as wpipe,
            tc.tile_pool(name="psum", bufs=4, space="PSUM") as psum,
        ):
            mask_t = consts.tile([P, 8], mybir.dt.int32)
            s0r_t = consts.tile([P, in_], mybir.dt.float16)
            s2_t = consts.tile([P, mid_k], mybir.dt.float32)
            s4_t = consts.tile([P, out_k], mybir.dt.float32)
            bias_t = consts.tile([P, out_k], mybir.dt.float32)
            neg_half = consts.tile([P, 1], mybir.dt.float32)
            for t, d in (
                (mask_t, mask_d),
                (s0r_t, s0r_d),
                (s2_t, s2_d),
                (s4_t, s4_d),
                (bias_t, bias_d),
            ):
                nc.gpsimd.dma_start(t[:], d[:])
            nc.vector.memset(neg_half[:], -0.5)

            # Warm the PE HAM clock gate with cheap junk matmuls while the
            # input pipeline fills, so the real stream starts at 2.4 GHz.
            junk = mask_t[:].bitcast(mybir.dt.float16)  # [P, 16] arbitrary bits
            warm_ps = psum.tile([P, 16], mybir.dt.float32, tag="warm")
            for _ in range(N_WARM):
                nc.tensor.matmul(warm_ps[:16, :], junk, junk, start=True, stop=True)

            _last_tr = [None]

            def load_bytes(bp_d, m, k_blocks, eng=None):
                kb = k_blocks * P // 8  # bytes per row
                byt = wpipe.tile([P, kb], mybir.dt.int32, tag="bytes", bufs=4)
                (eng or nc.sync).dma_start(byt[:], bp_d[m * P : (m + 1) * P, :])
                return byt

            def unpack_quarters(byt, k_blocks, scale_s0):
                """Unpack a loaded 128-row byte block into its transposed
                [P, k_blocks, P] weight tile, quarter by quarter.
                scale_s0: also multiply by the replicated scaling0 row."""
                wT = wpipe.tile([P, k_blocks, P], mybir.dt.float16, tag="wT", bufs=4)
                for c0 in range(0, k_blocks * P, uch):
                    nb = uch // 8
                    b0 = c0 // 8
                    masked = wpipe.tile([P, uch], mybir.dt.int32, tag="masked", bufs=3)
                    in0 = byt[:, b0 : b0 + nb][:, :, None].broadcast_to([P, nb, 8])
                    in1 = mask_t[:][:, None, :].broadcast_to([P, nb, 8])
                    nc.vector.tensor_tensor(
                        masked[:].rearrange("p (b j) -> p b j", j=8),
                        in0,
                        in1,
                        mybir.AluOpType.bitwise_and,
                    )
                    wq = wpipe.tile([P, uch], mybir.dt.float16, tag="wnat", bufs=4)
                    nc.scalar.activation(
                        wq[:], masked[:], Act.Sign, bias=neg_half[:, 0:1]
                    )
                    if scale_s0:
                        nc.gpsimd.tensor_tensor(
                            wq[:], wq[:], s0r_t[:, c0 : c0 + uch],
                            mybir.AluOpType.mult,
                        )
                    _last_tr[0] = nc.sync.dma_start_transpose(
                        wT[:, c0 // P : (c0 + uch) // P, :], wq[:]
                    ).ins
                return wT

            def unpack_wT(bp_d, m, k_blocks, scale_s0):
                return unpack_quarters(load_bytes(bp_d, m, k_blocks), k_blocks, scale_s0)

            # x.T in two batch halves: xH[h][p, k, r] = x[h*b/2 + r, 128k + p].
            # Band-split whole-half transposes read DRAM contiguously; no
            # scaling needed (scaling0 lives in W1).
            half = b // 2
            xH = [
                big.tile([P, in_k, half], mybir.dt.float16, tag=f"xT{h}", name=f"xh{h}")
                for h in range(2)
            ]

            def x_bands(h, after=None):
                # One full-width transpose per half: the DRAM read is fully
                # contiguous (whole rows), and 1024 xbar tiles keeps the DMA
                # semaphore threshold within the ISA field.
                tr = nc.sync.dma_start_transpose(
                    xH[h][:], x_d[h * half : (h + 1) * half, :]
                )
                if after is not None:
                    add_dep_helper(tr.ins, after, reason="x half-2 after startup wT")

            # Startup: prefetch byte blocks, transpose the first x half, then
            # unpack the first START_BLOCKS weight blocks, then the second x
            # half. The PE runs c0 passes of blocks 0..3 against the first x
            # half while the second is still transposing.
            SB = min(4, mid_k)
            x_bands(0)
            byts = [load_bytes(bp1_d, m, in_k, eng=nc.gpsimd) for m in range(SB)]
            wTs = [unpack_quarters(byts[m], in_k, True) for m in range(SB)]
            xh1_anchor = _last_tr[0]

            hT = big.tile([P, mid_k, b], mybir.dt.float16)

            def g1_pass(m, wT, c):
                ps = psum.tile([P, fd], mybir.dt.float32, tag="ps")
                for k in range(in_k):
                    nc.tensor.matmul(
                        ps[:],
                        wT[:, k, :],
                        xH[c][:, k, :],
                        start=(k == 0),
                        stop=(k == in_k - 1),
                    )
                nc.scalar.activation(
                    hT[:, m, c * fd : (c + 1) * fd],
                    ps[:],
                    Act.Copy,
                    scale=s2_t[:, m : m + 1],
                )

            # c-major startup over the first SB blocks; the second x half
            # transposes while the first-half passes run on the PE.
            for m in range(SB):
                g1_pass(m, wTs[m], 0)
            x_bands(1, after=xh1_anchor)
            for c in range(1, nbc):
                for m in range(SB):
                    g1_pass(m, wTs[m], c)

            # Unified steady loop: GEMM1 blocks SB.., then GEMM2 blocks, with
            # weight unpack prefetched two blocks ahead.
            n_blocks = mid_k + out_k

            def mk(jj):
                if jj >= n_blocks:
                    return None
                if jj < mid_k:
                    return unpack_wT(bp1_d, jj, in_k, True)
                return unpack_wT(bp3_d, jj - mid_k, mid_k, False)

            # GEMM2 output staging: groups of blocks buffered in the (dead)
            # x-half SBUF slots, stored with one DMA per group; the final
            # group is kept small so the tail store is short.
            yT_v = yT_d.rearrange("(g p) c -> p g c", p=P)
            ygroups = []
            _o = 0
            while _o < out_k:
                rem = out_k - _o
                if rem > 8:
                    n = 8
                elif rem > 2:
                    n = rem - 2
                else:
                    n = rem
                ygroups.append((_o, n))
                _o += n
            o2group = {}
            for gi_, (gs, gn) in enumerate(ygroups):
                for oo in range(gs, gs + gn):
                    o2group[oo] = (gi_, gs, gn)
            yt_g = None
            pend = [mk(SB), mk(SB + 1)]
            for j in range(SB, n_blocks):
                wT = pend.pop(0)
                pend.append(mk(j + 2))
                if j < mid_k:  # GEMM1 block
                    for c in range(nbc):
                        g1_pass(j, wT, c)
                else:  # GEMM2 block
                    o = j - mid_k
                    gi_, gstart, glen = o2group[o]
                    if o == gstart:
                        yt_g = big.tile(
                            [P, glen, b], mybir.dt.float16,
                            tag=f"xT{gi_ % 2}", name=f"ytg{o}",
                        )
                    for c in range(nbc):
                        ps = psum.tile([P, fd], mybir.dt.float32, tag="ps")
                        for k in range(mid_k):
                            nc.tensor.matmul(
                                ps[:],
                                wT[:, k, :],
                                hT[:, k, c * fd : (c + 1) * fd],
                                start=(k == 0),
                                stop=(k == mid_k - 1),
                            )
                        nc.scalar.activation(
                            yt_g[:, o - gstart, c * fd : (c + 1) * fd],
                            ps[:],
                            Act.Identity,
                            bias=bias_t[:, o : o + 1],
                            scale=s4_t[:, o : o + 1],
                        )
                    if o == gstart + glen - 1:
                        nc.sync.dma_start(
                            yT_v[:, gstart : gstart + glen, :], yt_g[:]
                        )

    nc.compile()
    return nc


def make_in_maps(x, scaling0, bp1, scaling2, bp3, scaling4, bias, n_cores=N_CORES):
    b_full, in_ = x.shape
    mid = scaling2.shape[0]
    out = scaling4.shape[0]
    b = b_full // n_cores

    mask = (1 << (7 - np.arange(8, dtype=np.int32)))[None, :].repeat(P, 0)
    mask = np.ascontiguousarray(mask.astype(np.int32))

    def pcol(v):
        return np.ascontiguousarray(v.astype(np.float32).reshape(-1, P).T)

    shared = {
        "bp1": np.ascontiguousarray(bp1.reshape(mid, in_ // 8)),
        "bp3": np.ascontiguousarray(bp3.reshape(out, mid // 8)),
        "mask": mask,
        "s0rep": np.ascontiguousarray(
            np.broadcast_to(scaling0.astype(np.float16)[None, :], (P, in_))
        ),
        "s2": pcol(scaling2),
        "s4": pcol(scaling4),
        "bias": pcol(bias),
    }
    return [
        {"x": np.ascontiguousarray(x[c * b : (c + 1) * b]), **shared}
        for c in range(n_cores)
    ]


_PROGRAM_CACHE = {}


def run(x, scaling0, bp1, scaling2, bp3, scaling4, bias, **spmd_kwargs):
    """Compile (cached) + run on 8 cores; returns (y, BassKernelResults)."""
    if "nc" not in _PROGRAM_CACHE:
        _PROGRAM_CACHE["nc"] = build_program()
    nc = _PROGRAM_CACHE["nc"]
    in_maps = make_in_maps(x, scaling0, bp1, scaling2, bp3, scaling4, bias)
    res = run_bass_kernel_spmd(nc, in_maps, core_ids=list(range(N_CORES)), **spmd_kwargs)
    b = x.shape[0] // N_CORES
    y = np.empty((x.shape[0], scaling4.shape[0]), dtype=np.float16)
    for c in range(N_CORES):
        y[c * b : (c + 1) * b] = res.results[c]["yT"].T
    return y, res


def kernel(x, scaling0, bp1, scaling2, bp3, scaling4, bias):
    y, _ = run(x, scaling0, bp1, scaling2, bp3, scaling4, bias)
    return y



# revision 8
# speedup vs baseline: 1.3388x; 1.3388x over previous
"""Trainium2 Bass kernel for DBFLinear:
    y = ((x * s0) @ unpack(bp1).T * s2) @ unpack(bp3).T * s4 + bias

Strategy (v2, fused weights): since W1/W3 are +/-1, precompute on device
    W13[i, o] = sum_m W1[m, i] * s2[m] * W3[o, m]        (build GEMM)
    y = x @ (s0[:, None] * W13) * s4 + bias              (main GEMM)
Total FLOPs drop from 2*B*IN*MID + 2*B*MID*OUT to
IN*MID*OUT (build, sharded) + 2*B*IN*OUT (main) -- 25% less PE work.

Sharding: each core owns 512 output columns (oc): it builds its W13
slice (contraction over m, stat = s2-scaled W3^T tiles, moving = W1
unpacked in natural layout -- no W1 transpose needed) and then runs the
main GEMM over the full batch (moving = host-transposed x^T windows).
The host assembles y from the 8 column shards (one transpose each).

LD_WEIGHTS amortization: hardware pays ~128 PE rows per stationary
load, unhidden.  Both GEMMs therefore reuse each loaded stationary
across multiple 512-row moving passes (2 in build, 4 in main; main
splits the i-contraction in halves with an SBUF fp16 partial-sum add
so 4 PSUM banks suffice per accumulation group).

SBUF: build-phase tiles live in a scoped pool released before the main
phase; x^T window buffers alternate between an early pool (prefetch
during build) and a pool carved from the released build zone.
"""

import sys

import numpy as np

sys.path.insert(0, "/opt/trn_rl_repo")

import concourse.bass as bass
import concourse.mybir as mybir
import concourse.tile as tile
from concourse import bacc
from concourse.bass_utils import run_bass_kernel_spmd

N_CORES = 8
B, IN, MID, OUT = 8192, 4096, 4096, 4096
P = 128
OC = OUT // N_CORES      # 512 output cols per core
N_OB = OC // P           # 4 stationary col-blocks
IK = IN // P             # 32 i-blocks
MK = MID // P            # 32 m-blocks
NCHUNK = 4               # build i-chunks of 1024
CH = IN // NCHUNK        # 1024
NWP = 4                  # main batch windows of 2048
WB = B // NWP            # 2048
NSP = WB // 512          # 4 spans of 512 per window
N_WARM = 800


def build_program():
    nc = bacc.Bacc(num_devices=N_CORES)
    f16, f32, i32 = mybir.dt.float16, mybir.dt.float32, mybir.dt.int32
    Act = mybir.ActivationFunctionType

    xT_d = nc.dram_tensor("xT", [IN, B], f16, kind="ExternalInput")
    bp1_d = nc.dram_tensor("bp1", [MID, IN // 8], i32, kind="ExternalInput")
    bp3_d = nc.dram_tensor("bp3", [OC, MID // 8], i32, kind="ExternalInput")
    mask_d = nc.dram_tensor("mask", [P, 8], i32, kind="ExternalInput")
    s0r_d = nc.dram_tensor("s0rep", [P, IN], f16, kind="ExternalInput")
    s2r_d = nc.dram_tensor("s2rep", [P, MID], f16, kind="ExternalInput")
    s4_d = nc.dram_tensor("s4", [P, N_OB], f32, kind="ExternalInput")
    bias_d = nc.dram_tensor("bias", [P, N_OB], f32, kind="ExternalInput")
    yT_d = nc.dram_tensor("yT", [OC, B], f16, kind="ExternalOutput")

    xTv = xT_d.rearrange("(h k p) b -> h p k b", p=P, k=IK // 2)  # [2,128,16,B]
    yv = yT_d.rearrange("(g p) b -> p g b", p=P)                  # [128,4,B]

    with tile.TileContext(nc) as tc:
        with (
            tc.tile_pool(name="consts", bufs=1) as consts,
            tc.tile_pool(name="wkP", bufs=1) as wkP,
            tc.tile_pool(name="xwpE", bufs=1) as xwpE,
            tc.tile_pool(name="psum", bufs=8, space="PSUM") as psum,
        ):
            mask_t = consts.tile([P, 8], i32)
            s4_t = consts.tile([P, N_OB], f32)
            bias_t = consts.tile([P, N_OB], f32)
            neg_half = consts.tile([P, 1], f32)
            for t, d in ((mask_t, mask_d), (s4_t, s4_d), (bias_t, bias_d)):
                nc.gpsimd.dma_start(t[:], d[:])
            nc.vector.memset(neg_half[:], -0.5)

            w13 = wkP.tile([P, IK, OC], f16, name="w13")    # 32KB/part

            _ps_n = [0]

            def ps_tile():
                _ps_n[0] += 1
                return psum.tile([P, 512], f32, tag="ps",
                                 name=f"ps{_ps_n[0]}")

            # Warm the PE HAM clock gate while W3 prep fills the pipeline.
            junk = mask_t[:].bitcast(f16)  # [P, 16]
            warm_ps = ps_tile()
            for _ in range(N_WARM):
                nc.tensor.matmul(warm_ps[:16, :16], junk, junk,
                                 start=True, stop=True)

            xw = {}

            def xw_load(wp, half, pool, tag):
                t = pool.tile([P, IK // 2, WB], f16, tag=tag, bufs=1,
                              name=f"xw_{wp}_{half}")
                nc.scalar.dma_start(
                    t[:], xTv[half, :, :, wp * WB:(wp + 1) * WB])
                xw[(wp, half)] = t

            with tc.tile_pool(name="wkB", bufs=1) as wk:
                s0r_t = wk.tile([P, IN], f16, name="s0r_t")
                s2r_t = wk.tile([P, MID], f16, name="s2r_t")
                nc.gpsimd.dma_start(s0r_t[:], s0r_d[:])
                nc.gpsimd.dma_start(s2r_t[:], s2r_d[:])
                w3sT = wk.tile([P, MK, OC], f16, name="w3sT")  # 32KB/part

                # -- W3 prep: unpack bp3 shard, scale by s2, transpose. --
                # q-major so w3sT m-blocks complete in consumption order.
                byt3 = [wk.tile([P, MID // 8], i32, tag="byt3", bufs=4,
                                name=f"byt3_{ob}")
                        for ob in range(N_OB)]
                for ob in range(N_OB):
                    nc.gpsimd.dma_start(byt3[ob][:],
                                        bp3_d[ob * P:(ob + 1) * P, :])
                for q in range(4):
                    for ob in range(N_OB):
                        masked = wk.tile([P, CH], i32, tag="masked", bufs=2,
                                         name=f"masked3_{q}_{ob}")
                        in0 = byt3[ob][:, q * 128:(q + 1) * 128][:, :, None] \
                            .broadcast_to([P, 128, 8])
                        in1 = mask_t[:][:, None, :].broadcast_to([P, 128, 8])
                        nc.vector.tensor_tensor(
                            masked[:].rearrange("p (b j) -> p b j", j=8),
                            in0, in1, mybir.AluOpType.bitwise_and)
                        wq3 = wk.tile([P, CH], f16, tag="wq3", bufs=2,
                                      name=f"wq3_{q}_{ob}")
                        nc.scalar.activation(wq3[:], masked[:], Act.Sign,
                                             bias=neg_half[:, 0:1])
                        nc.gpsimd.tensor_tensor(
                            wq3[:], wq3[:], s2r_t[:, q * CH:(q + 1) * CH],
                            mybir.AluOpType.mult)
                        nc.sync.dma_start_transpose(
                            w3sT[:, q * 8:(q + 1) * 8, ob * P:(ob + 1) * P],
                            wq3[:])

                # Prefetch the first x^T window during the build.
                xw_load(0, 0, xwpE, "xwE")

                # -- Build GEMM: W13^T chunks, scale by s0, transpose. --
                for c in range(NCHUNK):
                    psB = [ps_tile() for _ in range(8)]
                    for m in range(MK):
                        byt1 = wk.tile([P, CH // 8], i32, tag="byt1", bufs=4,
                                       name=f"byt1_{c}_{m}")
                        nc.gpsimd.dma_start(
                            byt1[:],
                            bp1_d[m * P:(m + 1) * P, c * 128:(c + 1) * 128])
                        masked = wk.tile([P, CH], i32, tag="masked1", bufs=2,
                                         name=f"masked1_{c}_{m}")
                        in0 = byt1[:][:, :, None].broadcast_to([P, 128, 8])
                        in1 = mask_t[:][:, None, :].broadcast_to([P, 128, 8])
                        nc.vector.tensor_tensor(
                            masked[:].rearrange("p (b j) -> p b j", j=8),
                            in0, in1, mybir.AluOpType.bitwise_and)
                        w1u = wk.tile([P, CH], f16, tag="w1u", bufs=3,
                                      name=f"w1u_{c}_{m}")
                        nc.scalar.activation(w1u[:], masked[:], Act.Sign,
                                             bias=neg_half[:, 0:1])
                        for ob in range(N_OB):
                            stat = w3sT[:, m, ob * P:(ob + 1) * P]
                            for w in range(2):
                                nc.tensor.matmul(
                                    psB[ob * 2 + w][:],
                                    stat,
                                    w1u[:, w * 512:(w + 1) * 512],
                                    start=(m == 0), stop=(m == MK - 1))
                    w13T = wk.tile([P, N_OB, CH], f16, tag="w13T", bufs=2,
                                   name=f"w13T_{c}")
                    for ob in range(N_OB):
                        for w in range(2):
                            nc.scalar.activation(
                                w13T[:, ob, w * 512:(w + 1) * 512],
                                psB[ob * 2 + w][:], Act.Copy)
                    nc.gpsimd.tensor_tensor(
                        w13T[:],
                        w13T[:],
                        s0r_t[:, c * CH:(c + 1) * CH][:, None, :]
                        .broadcast_to([P, N_OB, CH]),
                        mybir.AluOpType.mult)
                    for ob in range(N_OB):
                        nc.sync.dma_start_transpose(
                            w13[:, c * 8:(c + 1) * 8, ob * P:(ob + 1) * P],
                            w13T[:, ob, :])

            # ---- Main GEMM over x^T windows. ----
            with tc.tile_pool(name="wkM", bufs=1) as wkM:
                xw_load(0, 1, wkM, "xwL")
                for wp in range(NWP):
                    ypart = wkM.tile([P, N_OB, WB], f16, tag="ypart", bufs=1,
                                     name=f"ypart_{wp}")
                    for half in range(2):
                        k_idx = wp * 2 + half
                        nxt = (wp, 1) if half == 0 else (wp + 1, 0)
                        if nxt[0] < NWP and nxt not in xw:
                            npool, ntag = ((xwpE, "xwE") if (nxt[0] * 2 + nxt[1]) % 2 == 0
                                           else (wkM, "xwL"))
                            xw_load(nxt[0], nxt[1], npool, ntag)
                        xt = xw[(wp, half)]
                        for ob in range(N_OB):
                            psM = [ps_tile() for _ in range(NSP)]
                            for k in range(IK // 2):
                                stat = w13[:, half * 16 + k,
                                           ob * P:(ob + 1) * P]
                                for sp in range(NSP):
                                    nc.tensor.matmul(
                                        psM[sp][:],
                                        stat,
                                        xt[:, k, sp * 512:(sp + 1) * 512],
                                        start=(k == 0),
                                        stop=(k == IK // 2 - 1))
                            if half == 0:
                                for sp in range(NSP):
                                    nc.scalar.activation(
                                        ypart[:, ob, sp * 512:(sp + 1) * 512],
                                        psM[sp][:], Act.Identity,
                                        bias=bias_t[:, ob:ob + 1],
                                        scale=s4_t[:, ob:ob + 1])
                            else:
                                ystage = wkM.tile([P, WB], f16, tag="ystage",
                                                  bufs=2,
                                                  name=f"ystage_{wp}_{ob}")
                                for sp in range(NSP):
                                    ytmp = wkM.tile([P, 512], f16, tag="ytmp",
                                                    bufs=2,
                                                    name=f"ytmp_{wp}_{ob}_{sp}")
                                    nc.scalar.activation(
                                        ytmp[:], psM[sp][:], Act.Copy,
                                        scale=s4_t[:, ob:ob + 1])
                                    nc.vector.tensor_tensor(
                                        ystage[:, sp * 512:(sp + 1) * 512],
                                        ytmp[:],
                                        ypart[:, ob, sp * 512:(sp + 1) * 512],
                                        mybir.AluOpType.add)
                                nc.sync.dma_start(
                                    yv[:, ob, wp * WB:(wp + 1) * WB],
                                    ystage[:])

    nc.compile()
    return nc


def make_in_maps(x, scaling0, bp1, scaling2, bp3, scaling4, bias,
                 n_cores=N_CORES):
    mask = (1 << (7 - np.arange(8, dtype=np.int32)))[None, :].repeat(P, 0)
    mask = np.ascontiguousarray(mask.astype(np.int32))

    def pcol(v):
        return np.ascontiguousarray(v.astype(np.float32).reshape(-1, P).T)

    xT = np.ascontiguousarray(np.asarray(x, np.float16).T)
    bp1 = np.ascontiguousarray(bp1.reshape(MID, IN // 8))
    bp3 = np.ascontiguousarray(bp3.reshape(OUT, MID // 8))
    s0rep = np.ascontiguousarray(
        np.broadcast_to(scaling0.astype(np.float16)[None, :], (P, IN)))
    s2rep = np.ascontiguousarray(
        np.broadcast_to(scaling2.astype(np.float16)[None, :], (P, MID)))
    shared = {"xT": xT, "bp1": bp1, "mask": mask,
              "s0rep": s0rep, "s2rep": s2rep}
    maps = []
    for c in range(n_cores):
        sl = slice(c * OC, (c + 1) * OC)
        maps.append({
            "bp3": np.ascontiguousarray(bp3[sl]),
            "s4": pcol(scaling4[sl]),
            "bias": pcol(bias[sl]),
            **shared,
        })
    return maps


_PROGRAM_CACHE = {}


def run(x, scaling0, bp1, scaling2, bp3, scaling4, bias, **spmd_kwargs):
    """Compile (cached) + run on 8 cores; returns (y, BassKernelResults)."""
    if "nc" not in _PROGRAM_CACHE:
        _PROGRAM_CACHE["nc"] = build_program()
    nc = _PROGRAM_CACHE["nc"]
    in_maps = make_in_maps(x, scaling0, bp1, scaling2, bp3, scaling4, bias)
    res = run_bass_kernel_spmd(nc, in_maps, core_ids=list(range(N_CORES)),
                               **spmd_kwargs)
    y = np.empty((x.shape[0], scaling4.shape[0]), dtype=np.float16)
    for c in range(N_CORES):
        y[:, c * OC:(c + 1) * OC] = res.results[c]["yT"].T
    return y, res


def kernel(x, scaling0, bp1, scaling2, bp3, scaling4, bias):
    y, _ = run(x, scaling0, bp1, scaling2, bp3, scaling4, bias)
    return y
